# revision 47
# baseline (speedup 1.0000x reference)
"""BiasedMHA Trainium2 kernel (v4).

B=4, N=1024, FEAT=512, H=8 MHA with additive bias + bool mask, softmax over
keys, output projection. 8 cores: core c = batch c//2, head-group c%2
(4 heads = 2 pairs). Host sums the two per-batch partials and adds
bo + bv@Wo.T.

The kernel is ACT(exp)-paced (32 x [128,1024] exp ~ 36us), so everything
else is scheduled to hide under that stream:
  - per (pair, run, key-chunk): two score matmuls (head A rows 0-63, head B
    rows 64-127 - disjoint PE row groups, run concurrently) fill one
    2-bank PSUM tile; ONE exp covers both heads.
  - PV lags 2 chunks so the st->exp->mul chain never stalls the PE.
  - projections for the next pair + out-projection blocks are PE filler
    inside the attention stream; they share the st-tile PSUM ring so pv
    gets 4 rotating banks (no run-boundary stalls).
  - tails (denominator 1/x + normalize) are emitted split-phase TWO CHUNKS
    INTO THE NEXT RUN so they never head-of-line-block the ACT/DVE queues:
    phase 1 at jc==1 (dcol copy, den2 shift-DMA, ln, exp(-x)), phase 2 at
    jc==3 (reciprocal broadcast-DMA down 64 partitions, DVE normalize).
  - inputs: small first-dependency DMAs (wk, per-chunk nd) on the sync
    queue; wv + expb supertiles on the gpsimd queue (two parallel DMA
    streams), expb tiles dispatched lazily 2 chunks ahead of use.
  - expb = where(mask, 0, exp(bias)) f16 host-precomputed; exp(s+b) =
    exp(s)*expb. k-bias dropped (softmax-invariant); bq folded into QT.
"""

import numpy as np

import concourse.bass as bass
import concourse.mybir as mybir
import concourse.tile as tile
from concourse import bacc
from concourse.bass_utils import run_bass_kernel_spmd

_orig_get_tables = bacc.get_activation_tables


def _one_table(arch):
    t = _orig_get_tables(arch)
    return {k: (v if k == "natural_log_exp_and_others" else set())
            for k, v in t.items()}


bacc.get_activation_tables = _one_table

B, N, FEAT, H = 4, 1024, 512, 8
HD = FEAT // H          # 64
SCALE = HD ** -0.5
N_CORES = 8
HL = 4                  # local heads per core (2 pairs)
NJC = N // 128          # 8 key chunks
NFC = FEAT // 128       # 4 contraction chunks

F32 = mybir.dt.float32
F16 = mybir.dt.float16
AF = mybir.ActivationFunctionType

_CACHE = {}


def _build():
    nc = bacc.Bacc("TRN2", target_bir_lowering=False, debug=False)

    wk_d = nc.dram_tensor("wk_d", [128, 1024], F16, kind="ExternalInput").ap()
    wq_d = nc.dram_tensor("wq_d", [128, 1024], F16, kind="ExternalInput").ap()
    wv_d = nc.dram_tensor("wv_d", [128, 1024], F16, kind="ExternalInput").ap()
    wo_d = nc.dram_tensor("wo_d", [64, 2048], F16, kind="ExternalInput").ap()
    # nd_d[r] = ndata[b].T cols r*512.., fc blocks side by side (4KB rows)
    nd_d = nc.dram_tensor("nd_d", [2, 128, 2048], F16,
                          kind="ExternalInput").ap()
    bq2 = nc.dram_tensor("bq2", [128, 2], F32, kind="ExternalInput").ap()
    # ebq[t, q, p, (jc%2)*2048 + r*1024 + h2*512 + i2], q = jc//2
    ebq_d = nc.dram_tensor("ebq", [2, 4, 128, 4096], F16,
                           kind="ExternalInput").ap()
    # out[h4, p, q*512+x] = result row (h4*4+q)*128+p  (4KB DMA rows)
    out = nc.dram_tensor("out", [2, 128, 2048], F16,
                         kind="ExternalOutput").ap()

    with tile.TileContext(nc) as tc:
        with (
            tc.tile_pool(name="persist", bufs=1) as persist,
            tc.tile_pool(name="est", bufs=2) as est_pool,
            tc.tile_pool(name="etp", bufs=4) as et_pool,
            tc.tile_pool(name="tailp", bufs=2) as tailp,
            tc.tile_pool(name="outcp", bufs=2) as outcp,
            tc.tile_pool(name="ps_st", bufs=3, space="PSUM") as ps_st,
            tc.tile_pool(name="ps_pv", bufs=1, space="PSUM") as ps_pv,
        ):
            wk_sb = persist.tile([128, 1024], F16, tag="wk", name="wk")
            wq_sb = persist.tile([128, 1024], F16, tag="wq", name="wq")
            wv_sb = persist.tile([128, 1024], F16, tag="wv", name="wv")
            wo_sb = persist.tile([64, 2048], F16, tag="wo", name="wo")
            nd_sb = [persist.tile([128, 2048], F16, tag=f"nd{r}",
                                  name=f"nd{r}") for r in range(2)]
            ones_sb = persist.tile([128, 64], F16, tag="ones", name="ones")
            bq_sb = persist.tile([128, 2], F32, tag="bq", name="bq")
            KT = [persist.tile([128, N], F16, tag=f"kt{t}", name=f"kt{t}")
                  for t in range(2)]
            QT = [persist.tile([128, N], F16, tag=f"qt{t}", name=f"qt{t}")
                  for t in range(2)]
            V = persist.tile([128, NJC * (HL * 65)], F16, tag="v", name="v")
            ebq = [[persist.tile([128, 4096], F16, tag=f"eb{t}_{q}",
                                 name=f"eb{t}_{q}")
                    for q in range(4)] for t in range(2)]
            OTn = [persist.tile([64, N], F16, tag=f"otn{h}", name=f"otn{h}")
                   for h in range(HL)]
            warm = persist.tile([1, 2], F32, tag="warm", name="warm")
            warm2 = persist.tile([1, 2], F16, tag="warm2", name="warm2")

            def eb_slice(t, jc, r):
                q, o = jc // 2, (jc % 2) * 2048
                return ebq[t][q][:, o + r * 1024:o + (r + 1) * 1024]

            # ---- input DMAs ----
            # all input DMAs on the sync queue, in consumption order
            nc.sync.dma_start(out=bq_sb, in_=bq2)
            nc.sync.dma_start(out=wk_sb, in_=wk_d)
            nc.sync.dma_start(out=nd_sb[0], in_=nd_d[0])
            nc.sync.dma_start(out=wv_sb, in_=wv_d)
            nc.sync.dma_start(out=wq_sb, in_=wq_d)
            nc.sync.dma_start(out=nd_sb[1], in_=nd_d[1])
            nc.sync.dma_start(out=ebq[0][0], in_=ebq_d[0, 0])
            nc.sync.dma_start(out=ebq[0][1], in_=ebq_d[0, 1])
            nc.sync.dma_start(out=wo_sb, in_=wo_d)
            nc.sync.dma_start(out=ebq[0][2], in_=ebq_d[0, 2])
            nc.sync.dma_start(out=ebq[0][3], in_=ebq_d[0, 3])
            nc.sync.dma_start(out=ebq[1][0], in_=ebq_d[1, 0])
            nc.gpsimd.memset(warm, 0.0)
            nc.gpsimd.memset(ones_sb, 1.0)
            scratch = persist.tile([128, 512], F16, tag="scr", name="scr")
            nc.gpsimd.memset(scratch, 0.5)
            nc.gpsimd.memset(
                V.rearrange("p (jc h x) -> p jc h x", h=HL, x=65)[:, :, :, 64:65],
                1.0,
            )

            nc.scalar.activation(warm2, warm, AF.Exp)

            # PE warm-up: ~4us of dummy matmuls with no DMA dependencies
            # flips the HAM clock gate to 8/8 before the real projections.
            # Alternate output banks so WAW deps don't serialize them.
            wps = ps_st.tile([128, 1024], F32, tag="st", name="wps")
            for w in range(10):
                nc.tensor.matmul(
                    wps[0:64, (w % 2) * 512:(w % 2) * 512 + 512],
                    scratch[0:1, 0:64], scratch[0:1, :],
                    start=True, stop=True,
                )

            # ---- projection pieces (emitted upfront or as PE filler) ----
            proj_state = {}

            def kq_half(t, r, which):
                w_sb = wk_sb if which == "k" else wq_sb
                key = f"{which}p{t}"
                if r == 0:
                    proj_state[key] = ps_st.tile(
                        [128, 1024], F32, tag="st", name=key)
                ps = proj_state[key]
                for fc in range(NFC):
                    nc.tensor.matmul(
                        ps[:, r * 512:(r + 1) * 512],
                        w_sb[:, fc * 256 + t * 128:fc * 256 + (t + 1) * 128],
                        nd_sb[r][:, fc * 512:(fc + 1) * 512],
                        start=(fc == 0), stop=(fc == NFC - 1),
                    )
                if r == 1:
                    if which == "k":
                        nc.vector.tensor_copy(KT[t], ps)
                    else:
                        nc.vector.tensor_scalar_add(
                            QT[t], ps, bq_sb[:, t:t + 1])

            def vp_half(q2, sub):
                # q2 in 0..3 covers jt = 2*q2 + sub
                jt = 2 * q2 + sub
                key = f"vp{q2}"
                if sub == 0:
                    proj_state[key] = ps_st.tile(
                        [128, 1024], F32, tag="st", name=key)
                ps = proj_state[key]
                r, jl = jt // 4, jt % 4
                for fc in range(NFC):
                    nc.tensor.matmul(
                        ps[:, sub * 512:sub * 512 + 256],
                        nd_sb[r][:, fc * 512 + jl * 128:fc * 512
                              + (jl + 1) * 128],
                        wv_sb[:, fc * 256:(fc + 1) * 256],
                        start=(fc == 0), stop=(fc == NFC - 1),
                    )
                if sub == 1:
                    nc.vector.tensor_copy(
                        V.rearrange("p (jc h x) -> p jc h x", h=HL, x=65)
                         [:, 2 * q2:2 * q2 + 2, :, 0:64],
                        ps.rearrange("p (s h x) -> p s h x", s=2, x=64)
                          [:, :, 0:HL, :],
                    )

            fcp4 = [outcp.tile([128, 2048], F16, tag=f"fcp4_{h4}",
                               name=f"fcp4_{h4}") for h4 in range(2)]

            def emit_outproj(it):
                # single block on the st-ring; result collected into a
                # [128,2048] half-tile, DMA'd once per half (4KB rows)
                h4, q = it // 4, it % 4
                fp = ps_st.tile([128, 1024], F32, tag="st", name=f"fp{it}")
                for h in range(HL):
                    nc.tensor.matmul(
                        fp[:, 0:512],
                        OTn[h][:, it * 128:(it + 1) * 128],
                        wo_sb[:, h * 512:(h + 1) * 512],
                        start=(h == 0), stop=(h == HL - 1),
                    )
                nc.vector.tensor_copy(
                    fcp4[h4][:, q * 512:(q + 1) * 512], fp[:, 0:512])
                if q == 3:
                    nc.sync.dma_start(out=out[h4], in_=fcp4[h4])

            # ---- tails: direct PSUM path, queue-local (no DMA hops) ----
            def tail_phase1(t, run, pv2):
                # ln straight off the PSUM denominator row (both heads in
                # one [1,1024] op), then 1/x = exp(-ln)
                lnq = tailp.tile([65, 1024], F32, tag="lnq",
                                 name=f"lnq{t}{run}")
                nc.scalar.activation(lnq[64:65, :], pv2[64:65, :], AF.Ln)
                recq = tailp.tile([65, 1024], F16, tag="recq",
                                  name=f"recq{t}{run}")
                nc.scalar.activation(recq[64:65, :], lnq[64:65, :], AF.Exp,
                                     scale=-1.0)
                return recq

            def tail_phase2(t, run, pv2, recq):
                # broadcast 1/den down 64 partitions (K=1 ones matmul),
                # then normalize into OTn
                h0 = 2 * t
                rbc2 = ps_st.tile([128, 1024], F32, tag="st",
                                  name=f"rbc{t}{run}")
                for hh in range(2):
                    nc.tensor.matmul(
                        rbc2[0:64, hh * 512:(hh + 1) * 512],
                        ones_sb[64:65, 0:64],
                        recq[64:65, hh * 512:(hh + 1) * 512],
                        start=True, stop=True,
                    )
                rbs2 = tailp.tile([64, 1024], F16, tag="rbs2",
                                  name=f"rbs{t}{run}")
                nc.vector.tensor_copy(rbs2, rbc2[0:64, :])
                for hh in range(2):
                    nc.vector.tensor_mul(
                        OTn[h0 + hh][:, run * 512:(run + 1) * 512],
                        pv2[0:64, hh * 512:(hh + 1) * 512],
                        rbs2[:, hh * 512:(hh + 1) * 512])

            # ---- attention run-pass ----
            # pending tail work from the previous run, flushed at jc 1 / 3
            pend_tail = {}

            def emit_pair(t, run_fillers):
                h0 = 2 * t
                for run in range(2):
                    fillers = run_fillers[run]
                    pv2 = ps_pv.tile([65, 1024], F32, tag="pv",
                                     name=f"pv{t}{run}")
                    pend = []
                    for jc in range(NJC):
                        # fillers go FIRST: a filler that completes a PSUM
                        # ring slot's readers must precede the st matmul that
                        # reuses the slot, or the PE deadlocks on itself.
                        if jc == 2 and "p2" in pend_tail:
                            pt, pr, ppv, prec = pend_tail.pop("p2")
                            tail_phase2(pt, pr, ppv, prec)
                        if fillers:
                            f = fillers.pop(0)
                            if f is not None:
                                f()
                        st = ps_st.tile([128, 1024], F32, tag="st",
                                        name=f"st{t}_{run}_{jc}")
                        for hh in range(2):
                            po = 64 * hh
                            nc.tensor.matmul(
                                st[:, hh * 512:(hh + 1) * 512],
                                KT[t][po:po + 64, jc * 128:(jc + 1) * 128],
                                QT[t][po:po + 64, run * 512:(run + 1) * 512],
                                start=True, stop=True,
                            )
                        est = est_pool.tile([128, 1024], F16, tag="est",
                                            name=f"es{t}_{run}_{jc}")
                        nc.scalar.activation(est, st, AF.Exp)
                        if jc == 1 and "p1" in pend_tail:
                            pt, pr, ppv = pend_tail.pop("p1")
                            pend_tail["p2"] = (pt, pr, ppv,
                                               tail_phase1(pt, pr, ppv))
                        et = et_pool.tile([128, 1024], F16, tag="et",
                                          name=f"et{t}_{run}_{jc}")
                        nc.vector.tensor_mul(et, est, eb_slice(t, jc, run))
                        # lazy dispatch of pair-1 expb supertiles during
                        # pair-0 run-0 (everything else goes upfront)
                        if t == 0 and run == 0 and jc in (0, 2, 4):
                            q = jc // 2 + 1
                            nc.sync.dma_start(out=ebq[1][q], in_=ebq_d[1, q])
                        pend.append((jc, et))
                        if len(pend) > 2:
                            pjc, pet = pend.pop(0)
                            for hh in range(2):
                                nc.tensor.matmul(
                                    pv2[0:65, hh * 512:(hh + 1) * 512],
                                    V[:, pjc * 260 + (h0 + hh) * 65:
                                       pjc * 260 + (h0 + hh) * 65 + 65],
                                    pet[:, hh * 512:(hh + 1) * 512],
                                    start=(pjc == 0), stop=(pjc == NJC - 1),
                                )
                    for pjc, pet in pend:
                        for hh in range(2):
                            nc.tensor.matmul(
                                pv2[0:65, hh * 512:(hh + 1) * 512],
                                V[:, pjc * 260 + (h0 + hh) * 65:
                                   pjc * 260 + (h0 + hh) * 65 + 65],
                                pet[:, hh * 512:(hh + 1) * 512],
                                start=(pjc == 0), stop=(pjc == NJC - 1),
                            )
                    pend_tail["p1"] = (t, run, pv2)

            fill00 = [lambda q2=q2, s=s: vp_half(q2, s)
                      for q2 in range(4) for s in range(2)]
            fill01 = [lambda r=r: kq_half(1, r, "k") for r in range(2)]
            fill01 += [lambda r=r: kq_half(1, r, "q") for r in range(2)]
            # out-proj fillers from jc>=4 (run-0 OTn complete after the
            # previous run's phase-2 normalize, emitted at jc==2)
            fill11 = [None, None, None, None]
            fill11 += [lambda it=it: emit_outproj(it) for it in range(4)]

            # upfront: K/Q projections for pair 0 (V rides as filler)
            for r in range(2):
                kq_half(0, r, "k")
            for r in range(2):
                kq_half(0, r, "q")

            emit_pair(0, {0: fill00, 1: fill01})
            emit_pair(1, {0: [], 1: fill11})

            # ---- final tail (no next run to defer into) ----
            pt, pr, ppv = pend_tail.pop("p1")
            recq = tail_phase1(pt, pr, ppv)
            tail_phase2(pt, pr, ppv, recq)
            for it in range(4, 8):
                emit_outproj(it)

    nc.compile()
    return nc


def _prep_inputs(ndata, attn_bias, attn_mask, Wq, bq, Wk, bk, Wv, bv, Wo, bo):
    ndata = np.asarray(ndata, dtype=np.float32)
    attn_bias = np.asarray(attn_bias, dtype=np.float32)
    attn_mask = np.asarray(attn_mask)
    Wq, Wk, Wv, Wo = (np.asarray(w, dtype=np.float32) for w in (Wq, Wk, Wv, Wo))
    bq, bv, bo = (np.asarray(v, dtype=np.float32) for v in (bq, bv, bo))

    ebf = np.where(attn_mask, np.float32(0.0),
                   np.exp(attn_bias)).astype(np.float16)  # [B, i, j, H]

    wqT = (Wq.T * SCALE).astype(np.float16)
    wkT = Wk.T.astype(np.float16)
    wvT = Wv.T.astype(np.float16)
    woT = Wo.T.astype(np.float16)

    in_maps = []
    for core in range(N_CORES):
        b, hg = core // 2, core % 2
        h0 = hg * HL
        cw = slice(h0 * HD, (h0 + HL) * HD)
        ndT_b = ndata[b].T.astype(np.float16)          # [512, 1024]

        def tile4(w):
            # [512, 256] -> [128, 4*256] (fc blocks side by side)
            o = np.empty((128, 1024), dtype=np.float16)
            for fc in range(NFC):
                o[:, fc * 256:(fc + 1) * 256] = w[fc * 128:(fc + 1) * 128]
            return o

        wk2 = tile4(wkT[:, cw])
        wq2 = tile4(wqT[:, cw])
        wv2 = tile4(wvT[:, cw])
        wo2 = np.empty((64, 2048), dtype=np.float16)
        for h in range(HL):
            wo2[:, h * 512:(h + 1) * 512] = \
                woT[(h0 + h) * HD:(h0 + h + 1) * HD, :]
        nd4 = np.empty((2, 128, 2048), dtype=np.float16)
        for r in range(2):
            for fc in range(NFC):
                nd4[r][:, fc * 512:(fc + 1) * 512] = \
                    ndT_b[fc * 128:(fc + 1) * 128, r * 512:(r + 1) * 512]
        bq2 = np.ascontiguousarray(
            (bq[h0 * HD:(h0 + HL) * HD] * SCALE).reshape(2, 128).T
        ).astype(np.float32)
        # ebq[t, q, p, (jc%2)*2048 + r*1024 + h2*512 + i2]
        a = ebf[b][:, :, h0:h0 + HL]                  # [1024 i, 1024 j, 4]
        a = a.reshape(2, 512, NJC, 128, 2, 2)         # [r, i2, jc, p, t, h2]
        a = a.transpose(4, 2, 3, 0, 5, 1)             # [t, jc, p, r, h2, i2]
        a = a.reshape(2, NJC, 128, 2048)
        a = a.reshape(2, 4, 2, 128, 2048).transpose(0, 1, 3, 2, 4)
        ebq_core = np.ascontiguousarray(a.reshape(2, 4, 128, 4096))
        in_maps.append({
            "wk_d": np.ascontiguousarray(wk2),
            "wq_d": np.ascontiguousarray(wq2),
            "wv_d": np.ascontiguousarray(wv2),
            "wo_d": np.ascontiguousarray(wo2),
            "nd_d": np.ascontiguousarray(nd4),
            "bq2": bq2,
            "ebq": ebq_core,
        })
    boe = (bo + bv @ Wo.T).astype(np.float32)
    return in_maps, boe


def kernel(ndata, attn_bias, attn_mask, Wq, bq, Wk, bk, Wv, bv, Wo, bo,
           _trace=False):
    if "nc" not in _CACHE:
        _CACHE["nc"] = _build()
    nc = _CACHE["nc"]
    in_maps, boe = _prep_inputs(ndata, attn_bias, attn_mask, Wq, bq, Wk, bk,
                                Wv, bv, Wo, bo)
    res = run_bass_kernel_spmd(nc, in_maps, list(range(N_CORES)), trace=_trace)
    _CACHE["last_res"] = res
    full = np.empty((B, N, FEAT), dtype=np.float32)
    for b in range(B):
        # out[h4, p, q*512+x] -> row (h4*4+q)*128+p
        o0 = res.results[2 * b]["out"].reshape(2, 128, 4, 512)
        o1 = res.results[2 * b + 1]["out"].reshape(2, 128, 4, 512)
        o = o0.astype(np.float32) + o1.astype(np.float32)
        full[b] = (o.transpose(0, 2, 1, 3).reshape(N, FEAT) + boe[None, :])
    return full
